# revision 40
# baseline (speedup 1.0000x reference)
"""Trainium2 Bass kernel for nn_LocalTransformer (4-layer transformer,
d=1024, 16 heads, dff=4096, seq=1024, batch=4, causal + 64-lookahead mask).

Sharding: 8 cores = 4 samples x 2 sequence halves; each core owns 512
tokens. Attention context is a relative window of 1152 positions.

fp8 (e4m3) acceleration: Q/K/V/Wo projections and attention run as fp8
DoubleRow matmuls (0.5 cycles/row); QK uses a stride-0 broadcast lhsT with
a zero second k-tile in the Q rhs. Boundary masking is applied by adding a
precomputed NEG mask tile into the score PSUM via an identity matmul
(replaces gpsimd affine_selects, keeping Pool free for the collective).
Probs are fp8 (scale 2^EP folded into the exp bias; numerator/denominator
scales cancel). The K/V exchange between half-pairs is one merged fp8
ReduceScatter per layer (own+peer staged, receiver subtracts own). FFN
(W1/W2) stays bf16. LN/residual stays fp32.
"""
import numpy as np

L, D, H, DFF, S, B = 4, 1024, 16, 4096, 1024, 4
HD = D // H  # 64
T = 512  # local tokens per core
WIN = 1152  # kv window positions (9 chunks of 128)
NC = 9
EPS = 1e-5
NEG = -30000.0
V_E = H * 65  # 1040: per head [V(64) | denominator-ones col]

# fp8 scale exponents
EWQ, EWK, EWV, EWO = 13, 10, 10, 11
EX0, EX, EQ, EK, EP, EO = 5, 4, 5, 5, 2, 5
EW1, EX1 = 11, 4
LN2_ = float(np.log(2.0))

# packed per-layer params: columns of a [128, 96] tile
PC_BQ, PC_BK, PC_BO, PC_B2 = 0, 8, 16, 24
PC_L1G, PC_L1B, PC_L2G, PC_L2B = 32, 40, 48, 56
PC_B1 = 64  # 32 cols

# merged collective layout (fp8 bytes == elements)
KHALF = D * T  # 524288: K block [1024, 512]
VHALF = T * V_E  # 532480: V block [512, 1040]
HALF = KHALF + VHALF  # 1056768 = 1032*1024

# mask tile slots: chunk -> slot index (each slot is a full 512-wide strip)
MASKS = {4: 0, 5: 1, 6: 2, 7: 3, 8: 4}
MASKW = 5 * 512

_CACHE = {}


def _build_program():
    import concourse.bass as bass
    import concourse.tile as tile
    from concourse import bacc, mybir
    from contextlib import ExitStack

    f32, bf16, f32r = mybir.dt.float32, mybir.dt.bfloat16, mybir.dt.float32r
    fp8 = mybir.dt.float8e4
    AF = mybir.ActivationFunctionType
    ALU = mybir.AluOpType
    DR = mybir.MatmulPerfMode.DoubleRow

    nc = bacc.Bacc("TRN2", target_bir_lowering=False, debug=False, num_devices=8)

    I = {}
    I["x0"] = nc.dram_tensor("x0", [D, T], f32r, kind="ExternalInput").ap()
    I["pb"] = nc.dram_tensor("pb", [NC, 128, 1], f32, kind="ExternalInput").ap()
    I["par"] = nc.dram_tensor("par", [L, 128, 96], f32, kind="ExternalInput").ap()
    I["wq_pan"] = nc.dram_tensor("wq_pan", [L, 8, 128, 8, 128], fp8, kind="ExternalInput").ap()
    I["wk_pan"] = nc.dram_tensor("wk_pan", [L, 8, 128, 8, 128], fp8, kind="ExternalInput").ap()
    I["wo_pan"] = nc.dram_tensor("wo_pan", [L, 8, 128, 8, 128], fp8, kind="ExternalInput").ap()
    I["w1_pan"] = nc.dram_tensor("w1_pan", [L, 32, 128, 8, 2, 128], fp8, kind="ExternalInput").ap()
    I["w2_pan"] = nc.dram_tensor("w2_pan", [L, 8, 4, 128, 8, 128], bf16, kind="ExternalInput").ap()
    I["wv_pan"] = nc.dram_tensor("wv_pan", [L, 8, 128, V_E], fp8, kind="ExternalInput").ap()
    I["wks"] = nc.dram_tensor("wks", [L, 8, 1, 128], bf16, kind="ExternalInput").ap()
    I["wqs"] = nc.dram_tensor("wqs", [L, 8, 1, 128], bf16, kind="ExternalInput").ap()
    I["wvs"] = nc.dram_tensor("wvs", [L, 1, V_E], bf16, kind="ExternalInput").ap()
    I["vbias"] = nc.dram_tensor("vbias", [L, 1, V_E], bf16, kind="ExternalInput").ap()
    I["borow"] = nc.dram_tensor("borow", [L, 1, D], bf16, kind="ExternalInput").ap()
    I["ones1"] = nc.dram_tensor("ones1", [1, 128], f32r, kind="ExternalInput").ap()
    I["oneso"] = nc.dram_tensor("oneso", [1, 128], f32r, kind="ExternalInput").ap()
    I["onesd"] = nc.dram_tensor("onesd", [128, 1], f32r, kind="ExternalInput").ap()
    I["onesrb"] = nc.dram_tensor("onesrb", [1, 512], bf16, kind="ExternalInput").ap()
    I["id128"] = nc.dram_tensor("id128", [128, 2, 128], fp8, kind="ExternalInput").ap()
    I["maskt"] = nc.dram_tensor("maskt", [128, MASKW], fp8, kind="ExternalInput").ap()
    y = nc.dram_tensor("y", [D, T], f32, kind="ExternalOutput").ap()

    rct = nc.dram_tensor("rct", [1, 512], f32, kind="Internal").ap()
    rsk_in, rsk_out, rsv_in, rsv_out = [], [], [], []
    for l in range(L):
        rsk_in.append(nc.dram_tensor(f"rski{l}", [2 * D, T], fp8, kind="Internal").ap())
        rsk_out.append(nc.dram_tensor(f"rsko{l}", [D, T], fp8, kind="Internal").ap())
        rsv_in.append(nc.dram_tensor(f"rsvi{l}", [2 * T, V_E], fp8, kind="Internal").ap())
        rsv_out.append(nc.dram_tensor(f"rsvo{l}", [T, V_E], fp8, kind="Internal").ap())

    RG = [[0, 1], [2, 3], [4, 5], [6, 7]]

    with tile.TileContext(nc) as tc, ExitStack() as ctx:
        pers = ctx.enter_context(tc.tile_pool(name="pers", bufs=1))
        X = [pers.tile([128, T], f32r, tag=f"X{i}", name=f"X{i}") for i in range(8)]
        XD1 = [pers.tile([128, 2 * T], fp8, tag=f"XD1{i}", name=f"XD1{i}") for i in range(4)]
        XDD1 = [pers.tile([128, 2 * T], fp8, tag=f"XDD1{i}", name=f"XDD1{i}") for i in range(4)]
        X2 = [pers.tile([128, T], f32r, tag=f"X2{i}", name=f"X2{i}") for i in range(8)]
        XD = [pers.tile([128, 2 * T], fp8, tag=f"XD{i}", name=f"XD{i}") for i in range(4)]
        OPD = [pers.tile([128, 2 * T], fp8, tag=f"OPD{i}", name=f"OPD{i}") for i in range(4)]
        OST = [pers.tile([128, T], fp8, tag=f"OST{i}", name=f"OST{i}") for i in range(8)]
        Q = [pers.tile([128, 2 * T], fp8, tag=f"Q{i}", name=f"Qt{i}") for i in range(8)]
        KH = [pers.tile([128, WIN], fp8, tag=f"KH{i}", name=f"KHt{i}") for i in range(8)]
        # V pair tiles: VTP[p] holds window chunks (2p, 2p+1); VT8 single
        VTP = [pers.tile([128, 2 * V_E], fp8, tag=f"VTP{i}", name=f"VTPt{i}") for i in range(4)]
        VT8 = pers.tile([128, V_E], fp8, tag="VT8", name="VT8t")
        AVL = [pers.tile([128, T], bf16, tag=f"AVL{i}", name=f"AVLt{i}") for i in range(16)]
        HT = [pers.tile([128, T], bf16, tag=f"HT{i}", name=f"HTt{i}") for i in range(32)]
        ones1 = pers.tile([1, 128], f32r, tag="ones1", name="ones1t")
        oneso_t = pers.tile([1, 128], f32r, tag="oneso", name="onesot")
        onesd_t = pers.tile([128, 1], f32r, tag="onesd", name="onesdt")
        onesrb_t = pers.tile([1, T], bf16, tag="onesrb", name="onesrbt")
        id_t = pers.tile([128, 2, 128], fp8, tag="id128", name="id128t")
        mask_t = pers.tile([128, MASKW], fp8, tag="maskt", name="masktt")
        pb_t = [pers.tile([128, 1], f32, tag=f"pb{i}", name=f"pbt{i}") for i in range(NC)]
        RC = [pers.tile([128, 1], f32, tag=f"RC{i}", name=f"RCt{i}") for i in range(4)]
        ZC = pers.tile([128, 1], f32, tag="ZC", name="ZCt")

        wp = ctx.enter_context(tc.tile_pool(name="wp", bufs=4))  # fp8 [128,8,128] panels
        vwp = ctx.enter_context(tc.tile_pool(name="vwp", bufs=12))  # V weight pair panels
        w1p = ctx.enter_context(tc.tile_pool(name="w1p", bufs=4))  # bf16 W1/W2 panels
        smw = ctx.enter_context(tc.tile_pool(name="smw", bufs=2))  # small weights
        pp = ctx.enter_context(tc.tile_pool(name="pp", bufs=2, space="PSUM"))
        pp2 = ctx.enter_context(tc.tile_pool(name="pp2", bufs=2, space="PSUM"))
        pav = ctx.enter_context(tc.tile_pool(name="pav", bufs=2, space="PSUM"))
        prA = ctx.enter_context(tc.tile_pool(name="prA", bufs=8))  # probs pairs fp8
        prB = ctx.enter_context(tc.tile_pool(name="prB", bufs=2))  # probs singles fp8
        tps = ctx.enter_context(tc.tile_pool(name="tps", bufs=4))  # [128,T] f32
        tpr = ctx.enter_context(tc.tile_pool(name="tpr", bufs=2))  # [128,T] f32r/bf16
        fx = ctx.enter_context(tc.tile_pool(name="fx", bufs=2))  # fixup fp8 loads
        sm = ctx.enter_context(tc.tile_pool(name="sm", bufs=2))  # [1,T] smalls
        rhp = ctx.enter_context(tc.tile_pool(name="rhp", bufs=1))  # [1,T] recip
        pcp = ctx.enter_context(tc.tile_pool(name="pcp", bufs=3))  # params [128,96]

        r = lambda ap: ap.bitcast(f32r)
        q32 = lambda ap: ap.bitcast(f32)
        pair = lambda ap: ap.rearrange("p (a b) -> p a b", a=2)

        def bc2(ap, part, w):
            # [part, W] -> stride-0 [part, 2, W]
            return ap.rearrange("p (o b) -> p o b", o=1).to_broadcast((part, 2, w))

        for i in range(8):
            eng = nc.sync if i % 2 == 0 else nc.scalar
            eng.dma_start(out=X[i][:], in_=I["x0"][i * 128 : (i + 1) * 128, :])
        nc.scalar.dma_start(out=ones1[:], in_=I["ones1"][:])
        nc.scalar.dma_start(out=oneso_t[:], in_=I["oneso"][:])
        nc.scalar.dma_start(out=onesd_t[:], in_=I["onesd"][:])
        nc.scalar.dma_start(out=onesrb_t[:], in_=I["onesrb"][:])
        nc.scalar.dma_start(out=id_t[:], in_=I["id128"][:])
        nc.scalar.dma_start(out=mask_t[:], in_=I["maskt"][:])
        for i in range(NC):
            nc.scalar.dma_start(out=pb_t[i][:], in_=I["pb"][i])
        for i in range(8):
            # XD8 of x0 for layer-0 projections (before the memset burst so
            # the first K projection isn't queued behind it on Pool)
            nc.gpsimd.tensor_scalar_mul(
                XD[i // 2][:, (i % 2) * T : (i % 2 + 1) * T], q32(X[i][:]),
                float(2.0 ** EX0),
            )
        nc.gpsimd.memset(ZC[:], 0.0)
        for i in range(8):
            nc.gpsimd.memset(KH[i][:], 0.0)
            nc.gpsimd.memset(Q[i][:, T : 2 * T], 0.0)
        for i in range(4):
            nc.gpsimd.memset(VTP[i][:], 0.0)
        nc.gpsimd.memset(VT8[:], 0.0)

        def stage_k(lx):
            with tc.high_priority():
                for ec in range(8):
                    for half in (0, D):
                        nc.gpsimd.dma_start(
                            out=rsk_in[lx][half + ec * 128 : half + (ec + 1) * 128, :],
                            in_=KH[ec][:, 512:1024],
                        )
            nc.gpsimd.collective_compute(
                "ReduceScatter", mybir.AluOpType.add, replica_groups=RG,
                ins=[rsk_in[lx][:]], outs=[rsk_out[lx][:]],
            )

        def stage_v(lx):
            with tc.high_priority():
                for tcx in range(4):
                    src = VTP[2 + tcx // 2][:, (tcx % 2) * V_E : (tcx % 2 + 1) * V_E]
                    for half in (0, T):
                        nc.gpsimd.dma_start(
                            out=rsv_in[lx][half + tcx * 128 : half + (tcx + 1) * 128, :],
                            in_=src,
                        )
            nc.gpsimd.collective_compute(
                "ReduceScatter", mybir.AluOpType.add, replica_groups=RG,
                ins=[rsv_in[lx][:]], outs=[rsv_out[lx][:]],
            )

        def ones_cols(tile_ap, eng=None):
            (eng or nc.gpsimd).memset(
                tile_ap.rearrange("p (h c) -> p h c", c=65)[:, :, 64:65], 1.0
            )

        par0 = pcp.tile([128, 96], f32, tag="par", name="par")
        nc.sync.dma_start(out=par0[:], in_=I["par"][0])
        partile = par0

        def wpanel(src5d, l, ec, q=None):
            pan = wp.tile([128, 8, 128], fp8, tag="wpan", name="wpan")
            (q or nc.sync).dma_start(out=pan[:], in_=src5d[l, ec])
            return pan

        def proj_dr(pan, dst_ps, last_stops=False):
            # dst_ps[:, 0:T] += sum_j pan[:,2j:2j+2,:]^T (DR) @ XD[j]
            for j in range(4):
                nc.tensor.matmul(
                    out=dst_ps, lhsT=pan[:, 2 * j : 2 * j + 2, :], rhs=pair(XD[j][:]),
                    start=(j == 0), stop=(last_stops and j == 3), perf_mode=DR,
                )

        for l in range(L):
            par = partile

            if l == 0:
                # ---------- K projection -> KH[:, 512:1024], stage ----------
                for ec in range(8):
                    pan = wpanel(I["wk_pan"], l, ec)
                    ps = pp.tile([128, T], f32, tag="ps", name="ps")
                    proj_dr(pan, ps[:], last_stops=True)
                    nc.scalar.activation(
                        KH[ec][:, 512:1024], ps[:], AF.Identity,
                        bias=par[:, PC_BK + ec : PC_BK + ec + 1],
                        scale=float(2.0 ** (EK - EWK - EX0)),
                    )
                stage_k(l)

                # ---------- V projection (token-major), stage ----------
                vb = smw.tile([1, V_E], bf16, tag="vbias", name="vbias")
                nc.sync.dma_start(out=vb[:], in_=I["vbias"][l])
                vpre0 = {}
                for s0, s1 in ((0, 512), (512, 1024), (1024, V_E)):
                    w = s1 - s0
                    for j in range(4):
                        vp = vwp.tile([128, 2, 512], fp8, tag="vpan", name="vpan")
                        nc.sync.dma_start(out=vp[:, 0, 0:w], in_=I["wv_pan"][l, 2 * j, :, s0:s1])
                        nc.sync.dma_start(out=vp[:, 1, 0:w], in_=I["wv_pan"][l, 2 * j + 1, :, s0:s1])
                        vpre0[(s0, j)] = vp
                for s0, s1 in ((0, 512), (512, 1024), (1024, V_E)):
                    w = s1 - s0
                    vpans = [vpre0[(s0, j)] for j in range(4)]
                    for tcx in range(4):
                        ps = pp.tile([128, T], f32, tag="ps", name="ps")
                        for j in range(4):
                            nc.tensor.matmul(
                                out=ps[:, 0:w],
                                lhsT=pair(XD[j][:])[:, :, tcx * 128 : (tcx + 1) * 128],
                                rhs=vpans[j][:, :, 0:w],
                                start=(j == 0), stop=False, perf_mode=DR,
                            )
                        nc.tensor.matmul(
                            out=ps[:, 0:w],
                            lhsT=onesrb_t[:, tcx * 128 : (tcx + 1) * 128],
                            rhs=vb[:, s0:s1],
                            start=False, stop=True,
                        )
                        nc.scalar.activation(
                            VTP[2 + tcx // 2][:, (tcx % 2) * V_E + s0 : (tcx % 2) * V_E + s1],
                            ps[:, 0:w], AF.Copy,
                            scale=float(2.0 ** (EK - EWV - EX0)),
                        )
                for p in (2, 3):
                    ones_cols(VTP[p][:])
                stage_v(l)

                # ---------- Q projection ----------
                for ec in range(8):
                    pan = wpanel(I["wq_pan"], l, ec)
                    ps = pp.tile([128, T], f32, tag="ps", name="ps")
                    proj_dr(pan, ps[:], last_stops=True)
                    nc.scalar.activation(
                        Q[ec][:, 0:T], ps[:], AF.Identity,
                        bias=par[:, PC_BQ + ec : PC_BQ + ec + 1],
                        scale=float(2.0 ** (EQ - EWQ - EX0)),
                    )

            # ---------- attention: software-pipelined QK/exp ahead of AV ----------
            LOOKU = 8
            ESC = float(2.0 ** (-(EQ + EK)))

            def attn_pipeline(units, close_head):
                from collections import deque

                work = [(h, ui) for h in range(16) for ui in range(len(units))]
                nu = len(units)
                pend = deque()
                avs = {}

                def drain_one():
                    h2, ui2, pt2, single2 = pend.popleft()
                    if ui2 == 0:
                        avs[h2] = pav.tile([128, T], f32, tag="av", name="av")
                    if single2 is not True:
                        off2 = single2
                        vp_i = units[ui2][0] // 2
                        nc.tensor.matmul(
                            out=avs[h2][0:65, off2:T],
                            lhsT=pair(VTP[vp_i][:])[:, :, h2 * 65 : h2 * 65 + 65],
                            rhs=pair(pt2[:])[:, :, off2:T],
                            start=(ui2 == 0), stop=(ui2 == nu - 1), perf_mode=DR,
                        )
                    else:
                        nc.tensor.matmul(
                            out=avs[h2][0:65, 448:512],
                            lhsT=VT8[:, h2 * 65 : h2 * 65 + 65],
                            rhs=pt2[:, 448:512],
                            start=False, stop=(ui2 == nu - 1),
                        )
                    if ui2 == nu - 1:
                        close_head(h2, avs.pop(h2))

                for h, ui in work:
                    unit = units[ui]
                    par_, kc = h % 2, h // 2
                    rows = slice(par_ * 64, par_ * 64 + 64)
                    if len(unit) == 2:
                        off = 192 if unit[0] == 6 else 0
                        w = T - off
                        sc = pp2.tile([128, 2 * T], f32, tag="ps2", name="ps2")
                        pt = prA.tile([128, 2 * T], fp8, tag="probs2", name="probs2")
                        for k, c in enumerate(unit):
                            nc.tensor.matmul(
                                out=sc[:, k * T + off : (k + 1) * T],
                                lhsT=bc2(KH[kc][rows, c * 128 : (c + 1) * 128], 64, 128),
                                rhs=pair(Q[kc][rows, :])[:, :, off:T],
                                start=True, stop=(c not in MASKS),
                                perf_mode=DR,
                            )
                            if c in MASKS:
                                ms = MASKS[c] * T
                                nc.tensor.matmul(
                                    out=sc[:, k * T + off : (k + 1) * T],
                                    lhsT=id_t[:],
                                    rhs=bc2(mask_t[:, ms + off : ms + T], 128, T - off),
                                    start=False, stop=True, perf_mode=DR,
                                )
                        nc.scalar.activation(
                            pair(pt[:])[:, :, off:T], pair(sc[:])[:, :, off:T],
                            AF.Exp, bias=pb_t[unit[0]][:], scale=ESC,
                        )
                        pend.append((h, ui, pt, off))
                    else:
                        c = unit[0]  # chunk 8
                        sc = pp.tile([128, T], f32, tag="ps", name="ps")
                        pt = prB.tile([128, T], fp8, tag="probs", name="probs")
                        nc.tensor.matmul(
                            out=sc[:, 448:512],
                            lhsT=KH[kc][rows, 1024:1152],
                            rhs=Q[kc][rows, 448:512],
                            start=True, stop=False,
                        )
                        ms = MASKS[8] * T
                        nc.tensor.matmul(
                            out=sc[:, 448:512],
                            lhsT=id_t[:],
                            rhs=bc2(mask_t[:, ms + 448 : ms + 512], 128, 64),
                            start=False, stop=True, perf_mode=DR,
                        )
                        nc.scalar.activation(
                            pt[:, 448:512], sc[:, 448:512], AF.Exp,
                            bias=pb_t[8][:], scale=ESC,
                        )
                        pend.append((h, ui, pt, True))
                    if len(pend) > LOOKU:
                        drain_one()
                while pend:
                    drain_one()

            attn_pipeline(
                [(4, 5), (6, 7)],
                lambda h, av: nc.vector.tensor_copy(AVL[h][0:65, :], av[0:65, :]),
            )

            # ---------- K/V fixup (consumes RS results) ----------
            for ec in range(8):
                ka = fx.tile([128, T], fp8, tag="fxk", name="fxk")
                nc.sync.dma_start(out=ka[:], in_=rsk_out[l][ec * 128 : (ec + 1) * 128, :])
                nc.vector.tensor_sub(KH[ec][:, 0:512], ka[:], KH[ec][:, 512:1024])
                nc.vector.tensor_sub(
                    KH[ec][:, 1024:1088], ka[:, 0:64], KH[ec][:, 512:576]
                )
            va0 = None
            for tcx in range(4):
                va = fx.tile([128, V_E], fp8, tag="fxv", name="fxv")
                nc.sync.dma_start(
                    out=va[:], in_=rsv_out[l][tcx * 128 : (tcx + 1) * 128, :]
                )
                dst = VTP[tcx // 2][:, (tcx % 2) * V_E : (tcx % 2 + 1) * V_E]
                own = VTP[2 + tcx // 2][:, (tcx % 2) * V_E : (tcx % 2 + 1) * V_E]
                nc.vector.tensor_sub(dst, va[:], own)
                if tcx == 0:
                    va0 = va
            nc.vector.tensor_sub(
                VT8[0:64, :], va0[0:64, :], VTP[2][0:64, 0:V_E]
            )

            for p in (0, 1):
                ones_cols(VTP[p][:], nc.gpsimd)
            ones_cols(VT8[:], nc.gpsimd)

            # ---------- attention phase B: remote chunks + deferred combine ----------
            def close_b(h, av2):
                avt = tps.tile([128, T], f32, tag="t512", name="t512")
                nc.vector.tensor_add(avt[0:65, :], av2[0:65, :], AVL[h][0:65, :])
                rr = rhp.tile([1, T], f32r, tag="rh", name="rht")
                with nc.allow_low_precision(reason="f32r is fp32-width storage"):
                    nc.vector.reciprocal(rr[:], avt[64:65, :])
                bc = pp.tile([128, T], f32, tag="ps", name="ps")
                nc.tensor.matmul(
                    out=bc[0:64, :], lhsT=r(oneso_t[:, 0:64]), rhs=r(rr[:]),
                    start=True, stop=True,
                )
                dc = h // 2
                if h % 2 == 0:
                    nc.vector.tensor_mul(
                        OPD[dc // 2][0:64, (dc % 2) * T : (dc % 2 + 1) * T],
                        avt[0:64, :], bc[0:64, :],
                    )
                else:
                    nc.vector.tensor_mul(OST[dc][0:64, :], avt[0:64, :], bc[0:64, :])

            attn_pipeline([(0, 1), (2, 3), (8,)], close_b)
            for dc in range(8):
                nc.sync.dma_start(
                    out=OPD[dc // 2][64:128, (dc % 2) * T : (dc % 2 + 1) * T],
                    in_=OST[dc][0:64, :],
                )

            def ln_stats(src, make_kprep=False, pre=None):
                if pre is not None:
                    mu, ms = pre
                else:
                    mu = pp.tile([1, T], f32, tag="ps", name="ps")
                    ms = pp.tile([1, T], f32, tag="ps", name="ps")
                    for dc in range(8):
                        sq = tpr.tile([128, T], f32r, tag="sqr", name="sqr")
                        nc.gpsimd.tensor_mul(sq[:], q32(src[dc][:]), q32(src[dc][:]))
                        nc.tensor.matmul(
                            out=mu[:], lhsT=r(onesd_t[:]), rhs=r(src[dc][:]),
                            start=(dc == 0), stop=(dc == 7),
                        )
                        nc.tensor.matmul(
                            out=ms[:], lhsT=r(onesd_t[:]), rhs=r(sq[:]),
                            start=(dc == 0), stop=(dc == 7),
                        )
                mu_sb = sm.tile([1, T], f32r, tag="sm1", name="mu")
                nc.vector.tensor_copy(mu_sb[:], mu[:])
                MS = None
                if make_kprep:
                    MS = rhp.tile([1, 2 * T], bf16, tag="rh", name="mustd16")
                    nc.vector.tensor_copy(MS[:, 0:T], q32(mu_sb[:]))
                t2 = sm.tile([1, T], f32, tag="sm1", name="t2")
                nc.vector.tensor_mul(t2[:], q32(mu_sb[:]), q32(mu_sb[:]))
                mub = pp.tile([128, T], f32, tag="ps", name="ps")
                nc.tensor.matmul(
                    out=mub[:], lhsT=r(ones1[:]), rhs=r(mu_sb[:]), start=True, stop=True
                )
                var = sm.tile([1, T], f32, tag="sm1", name="var")
                nc.vector.tensor_sub(var[:], ms[:], t2[:])
                nc.vector.tensor_scalar_add(var[:], var[:], EPS)
                std = sm.tile([1, T], f32, tag="sm1", name="std")
                nc.scalar.sqrt(std[:], var[:])
                if make_kprep:
                    nc.vector.tensor_copy(MS[:, T : 2 * T], std[:])
                rstd = sm.tile([1, T], f32r, tag="sm1", name="rstd")
                with nc.allow_low_precision(reason="f32r is fp32-width storage"):
                    nc.vector.reciprocal(rstd[:], std[:])
                rsb = pp.tile([128, T], f32, tag="ps", name="ps")
                nc.tensor.matmul(
                    out=rsb[:], lhsT=r(ones1[:]), rhs=r(rstd[:]), start=True, stop=True
                )
                if make_kprep:
                    # per-token rstd columns via a DRAM bounce, pre-scaled for
                    # the fused-V epilogue
                    rsv = sm.tile([1, T], f32, tag="sm1", name="rsv")
                    nc.vector.tensor_scalar_mul(
                        rsv[:], q32(rstd[:]), float(2.0 ** (EK - EWV - EX))
                    )
                    nc.sync.dma_start(out=rct[:], in_=rsv[:])
                    for tcx in range(4):
                        nc.sync.dma_start(
                            out=RC[tcx][:],
                            in_=rct[0, tcx * 128 : (tcx + 1) * 128].rearrange(
                                "(c o) -> c o", o=1
                            ),
                        )
                return MS, mub, rsb

            def ln_norm(src, stats, gcol, bcol, dst, lpar, make_xb=False):
                _, mub, rsb = stats
                rsb_sb = tps.tile([128, T], f32, tag="t512", name="t512")
                nc.vector.tensor_copy(rsb_sb[:], rsb[:])
                for dc in range(8):
                    t = tps.tile([128, T], f32, tag="t512", name="t512")
                    nc.vector.tensor_sub(t[:], q32(src[dc][:]), mub[:])
                    t2b = tps.tile([128, T], f32, tag="t512", name="t512")
                    nc.vector.tensor_mul(t2b[:], t[:], rsb_sb[:])
                    nc.scalar.activation(
                        dst[dc][:], t2b[:], AF.Identity,
                        bias=lpar[:, bcol + dc : bcol + dc + 1],
                        scale=lpar[:, gcol + dc : gcol + dc + 1],
                    )
                    if make_xb:
                        x1 = XD1[dc // 2][:, (dc % 2) * T : (dc % 2 + 1) * T]
                        nc.gpsimd.tensor_scalar_mul(
                            x1, q32(dst[dc][:]), float(2.0 ** EX1)
                        )
                        nc.vector.scalar_tensor_tensor(
                            out=XDD1[dc // 2][:, (dc % 2) * T : (dc % 2 + 1) * T],
                            in0=q32(dst[dc][:]), scalar=float(2.0 ** EX1),
                            in1=x1, op0=ALU.mult, op1=ALU.subtract,
                        )

            def layernorm(src, gcol, bcol, dst, make_xb=False):
                ln_norm(src, ln_stats(src), gcol, bcol, dst, par, make_xb=make_xb)

            # ---------- Wo + residual + LN1 ----------
            borow_t = smw.tile([1, D], bf16, tag="vbias", name="borow")
            nc.sync.dma_start(out=borow_t[:], in_=I["borow"][l])
            for ec in range(8):
                pan = wp.tile([128, 8, 128], fp8, tag="wpan2", name="wpan2")
                nc.sync.dma_start(out=pan[:], in_=I["wo_pan"][l, ec])
                ps = pp.tile([128, T], f32, tag="ps", name="ps")
                for j in range(4):
                    nc.tensor.matmul(
                        out=ps[:], lhsT=pan[:, 2 * j : 2 * j + 2, :],
                        rhs=pair(OPD[j][:]),
                        start=(j == 0), stop=False, perf_mode=DR,
                    )
                nc.tensor.matmul(
                    out=ps[:], lhsT=borow_t[:, ec * 128 : (ec + 1) * 128],
                    rhs=onesrb_t[:], start=False, stop=True,
                )
                nc.vector.scalar_tensor_tensor(
                    out=X2[ec][:], in0=ps[:], scalar=float(2.0 ** (-(EO + EWO))),
                    in1=q32(X[ec][:]), op0=ALU.mult, op1=ALU.add,
                )
            layernorm(X2, PC_L1G, PC_L1B, X, make_xb=True)

            # ---------- FFN: W1 -> H (bf16), W2 accumulated in PSUM ----------
            for fc in range(32):
                pan = w1p.tile([128, 8, 2, 128], fp8, tag="w1pan", name="w1pan")
                nc.sync.dma_start(out=pan[:], in_=I["w1_pan"][l, fc])
                ps = pp.tile([128, T], f32, tag="ps", name="ps")
                for dc in range(8):
                    x1 = XD1[dc // 2][:, (dc % 2) * T : (dc % 2 + 1) * T]
                    nc.tensor.matmul(
                        out=ps[:], lhsT=pan[:, dc, :, :], rhs=bc2(x1, 128, T),
                        start=(dc == 0), stop=False, perf_mode=DR,
                    )
                for j in range(4):
                    nc.tensor.matmul(
                        out=ps[:],
                        lhsT=pan[:, 2 * j : 2 * j + 2, 0:1, :].rearrange(
                            "p a o b -> p (a o) b"
                        ),
                        rhs=pair(XDD1[j][:]),
                        start=False, stop=(j == 3), perf_mode=DR,
                    )
                nc.scalar.activation(
                    HT[fc][:], ps[:], AF.Relu,
                    bias=par[:, PC_B1 + fc : PC_B1 + fc + 1],
                    scale=float(2.0 ** (-(EW1 + EX1))),
                )
            for ec in range(8):
                ps = pp.tile([128, T], f32, tag="ps", name="ps")
                for g in range(4):
                    pan = w1p.tile([128, 8, 128], bf16, tag="w2pan", name="w2pan")
                    nc.sync.dma_start(out=pan[:], in_=I["w2_pan"][l, ec, g])
                    for k in range(8):
                        fc = g * 8 + k
                        nc.tensor.matmul(
                            out=ps[:], lhsT=pan[:, k, :], rhs=HT[fc][:],
                            start=(fc == 0), stop=(fc == 31),
                        )
                nc.vector.scalar_tensor_tensor(
                    out=X2[ec][:], in0=ps[:], scalar=par[:, PC_B2 + ec : PC_B2 + ec + 1],
                    in1=q32(X[ec][:]), op0=ALU.add, op1=ALU.add,
                )
            # ---------- LN2 + deferred next-layer K/V/Q projections ----------
            stats2 = ln_stats(X2, make_kprep=(l < L - 1))
            if l < L - 1:
                ln2 = l + 1
                par_next = pcp.tile([128, 96], f32, tag="par", name="par")
                nc.sync.dma_start(out=par_next[:], in_=I["par"][ln2])
                MS, _, rsb2 = stats2
                RSB = tpr.tile([128, T], bf16, tag="kb16", name="rsbk")
                nc.vector.tensor_copy(RSB[:], rsb2[:])
                for dc in range(8):
                    # XD8 of X2 for the next layer's projections
                    nc.gpsimd.tensor_scalar_mul(
                        XD[dc // 2][:, (dc % 2) * T : (dc % 2 + 1) * T],
                        q32(X2[dc][:]), float(2.0 ** EX),
                    )
                wkst = sm.tile([1, 8 * 128], bf16, tag="sm1", name="wkst")
                nc.sync.dma_start(
                    out=wkst[:], in_=I["wks"][ln2].rearrange("e o c -> o (e c)")
                )
                wqst = sm.tile([1, 8 * 128], bf16, tag="sm1", name="wqst")
                nc.sync.dma_start(
                    out=wqst[:], in_=I["wqs"][ln2].rearrange("e o c -> o (e c)")
                )

                def fproj(dst_write, pan_src, wsrow, bcolx, ec, escale):
                    pan = wpanel(pan_src, ln2, ec)
                    ps = pav.tile([128, T], f32, tag="av", name="av")
                    proj_dr(pan, ps[:])
                    nc.tensor.matmul(
                        out=ps[:], lhsT=wsrow[:, ec * 128 : (ec + 1) * 128],
                        rhs=MS[:, 0:T], start=False, stop=True,
                    )
                    ktmp = tpr.tile([128, T], bf16, tag="kb16", name="ktmp")
                    nc.vector.tensor_mul(ktmp[:], ps[:], RSB[:])
                    nc.scalar.activation(
                        dst_write, ktmp[:], AF.Identity,
                        bias=par_next[:, bcolx + ec : bcolx + ec + 1], scale=escale,
                    )

                for ec in range(8):
                    fproj(
                        KH[ec][:, 512:1024], I["wk_pan"], wkst, PC_BK, ec,
                        float(2.0 ** (EK - EWK - EX)),
                    )
                stage_k(ln2)

                for ec in range(8):
                    fproj(
                        Q[ec][:, 0:T], I["wq_pan"], wqst, PC_BQ, ec,
                        float(2.0 ** (EQ - EWQ - EX)),
                    )

                vb = smw.tile([1, V_E], bf16, tag="vbias", name="vbias")
                nc.sync.dma_start(out=vb[:], in_=I["vbias"][ln2])
                wvst = smw.tile([1, V_E], bf16, tag="vbias", name="wvst")
                nc.sync.dma_start(out=wvst[:], in_=I["wvs"][ln2])
                vpre = {}
                for s0, s1 in ((0, 512), (512, 1024), (1024, V_E)):
                    w = s1 - s0
                    for j in range(4):
                        vp = vwp.tile([128, 2, 512], fp8, tag="vpan", name="vpan")
                        nc.sync.dma_start(out=vp[:, 0, 0:w], in_=I["wv_pan"][ln2, 2 * j, :, s0:s1])
                        nc.sync.dma_start(out=vp[:, 1, 0:w], in_=I["wv_pan"][ln2, 2 * j + 1, :, s0:s1])
                        vpre[(s0, j)] = vp
                for s0, s1 in ((0, 512), (512, 1024), (1024, V_E)):
                    w = s1 - s0
                    vpans = [vpre[(s0, j)] for j in range(4)]
                    for tcx in range(4):
                        ps = pp.tile([128, T], f32, tag="ps", name="ps")
                        for j in range(4):
                            nc.tensor.matmul(
                                out=ps[:, 0:w],
                                lhsT=pair(XD[j][:])[:, :, tcx * 128 : (tcx + 1) * 128],
                                rhs=vpans[j][:, :, 0:w],
                                start=(j == 0), stop=False, perf_mode=DR,
                            )
                        nc.tensor.matmul(
                            out=ps[:, 0:w],
                            lhsT=MS[:, tcx * 128 : (tcx + 1) * 128],
                            rhs=wvst[:, s0:s1],
                            start=False, stop=False,
                        )
                        nc.tensor.matmul(
                            out=ps[:, 0:w],
                            lhsT=MS[:, T + tcx * 128 : T + (tcx + 1) * 128],
                            rhs=vb[:, s0:s1],
                            start=False, stop=True,
                        )
                        nc.scalar.activation(
                            VTP[2 + tcx // 2][:, (tcx % 2) * V_E + s0 : (tcx % 2) * V_E + s1],
                            ps[:, 0:w], AF.Identity,
                            bias=ZC[:], scale=RC[tcx][:],
                        )
                for p in (2, 3):
                    ones_cols(VTP[p][:])
                stage_v(ln2)

                ln_norm(X2, stats2, PC_L2G, PC_L2B, X, par, make_xb=False)
                partile = par_next
            else:
                ln_norm(X2, stats2, PC_L2G, PC_L2B, X, par, make_xb=False)

        for ec in range(8):
            nc.sync.dma_start(out=y[ec * 128 : (ec + 1) * 128, :], in_=q32(X[ec][:]))

    nc.compile()
    return nc


def _to_bf16(a):
    import ml_dtypes

    return np.asarray(a, np.float32).astype(ml_dtypes.bfloat16)


def _to_fp8(a, exp):
    import ml_dtypes

    x = np.asarray(a, np.float32) * (2.0 ** exp)
    x = np.clip(x, -224.0, 224.0)
    return x.astype(ml_dtypes.float8_e4m3)


def _host_prep(inputs):
    g = {}
    Wqkv = np.asarray(inputs["Wqkv"], np.float32)
    bqkv = np.asarray(inputs["bqkv"], np.float32)
    sc = 1.0 / np.sqrt(HD)
    par_bq_override, par_bk_override = [], []
    wvT = np.zeros((L, D, V_E), np.float32)
    vbias = np.zeros((L, 1, V_E), np.float32)
    wq = np.zeros((L, D, D), np.float32)
    wk = np.zeros((L, D, D), np.float32)
    for l in range(L):
        Wq, Wk, Wv = Wqkv[l, 0:D], Wqkv[l, D : 2 * D], Wqkv[l, 2 * D :]
        bv = bqkv[l, 2 * D :]
        wq[l] = Wq.T * sc
        wk[l] = Wk.T
        for h in range(H):
            off = h * 65
            wvT[l, :, off : off + 64] = Wv.T[:, h * 64 : h * 64 + 64]
            vbias[l, 0, off : off + 64] = bv[h * 64 : h * 64 + 64]

    def pan5(wT):  # [L, D, M] -> [L, M/128, 128, 8, 128]
        Lx, Dx, M = wT.shape
        return np.ascontiguousarray(
            wT.reshape(Lx, 8, 128, M // 128, 128).transpose(0, 3, 2, 1, 4)
        )

    woT = np.asarray(inputs["Wo"], np.float32).transpose(0, 2, 1)
    w1T = np.asarray(inputs["W1"], np.float32).transpose(0, 2, 1)
    w2T = np.asarray(inputs["W2"], np.float32).transpose(0, 2, 1)
    g2f = np.asarray(inputs["g2"], np.float32)
    be2f = np.asarray(inputs["be2"], np.float32)
    wks = np.zeros((L, 8, 1, 128), np.float32)
    wqs = np.zeros((L, 8, 1, 128), np.float32)
    wvs = np.zeros((L, 1, V_E), np.float32)
    for l in range(1, L):
        Wq, Wk, Wv = Wqkv[l, 0:D], Wqkv[l, D : 2 * D], Wqkv[l, 2 * D :]
        bq_ = bqkv[l, 0:D]
        bk_ = bqkv[l, D : 2 * D]
        bv_ = bqkv[l, 2 * D :]
        gp, bp = g2f[l - 1], be2f[l - 1]
        wq[l] = Wq.T * sc * gp[:, None]
        wk[l] = Wk.T * gp[:, None]
        wks[l] = -wk[l].sum(axis=0).reshape(8, 1, 128) * (2.0 ** (EWK + EX))
        wqs[l] = -wq[l].sum(axis=0).reshape(8, 1, 128) * (2.0 ** (EWQ + EX))
        par_bq_override.append((l, (Wq @ bp) * sc + bq_ * sc))
        par_bk_override.append((l, Wk @ bp + bk_))
        wvT[l] *= gp[:, None]
        bvf = Wv @ bp + bv_
        for h in range(H):
            off = h * 65
            vbias[l, 0, off : off + 64] = bvf[h * 64 : h * 64 + 64]
        wvs[l, 0] = -wvT[l].sum(axis=0) * (2.0 ** (EWV + EX))
    g["wks"] = _to_bf16(wks)
    g["wqs"] = _to_bf16(wqs)
    g["wvs"] = _to_bf16(wvs)
    g["wq_pan"] = _to_fp8(pan5(wq), EWQ)
    g["wk_pan"] = _to_fp8(pan5(wk), EWK)
    g["wo_pan"] = _to_fp8(pan5(woT), EWO)
    w1p5 = pan5(w1T)  # [L, 32, 128, 8, 128]
    w1hi = _to_fp8(w1p5, EW1)
    w1lo = _to_fp8(w1p5 * (2.0 ** EW1) - w1hi.astype(np.float32), 0)
    g["w1_pan"] = np.ascontiguousarray(
        np.stack([w1hi, w1lo], axis=4)
    )  # [L, 32, 128, 8, 2, 128] fp8
    w2p = np.empty((L, 8, 4, 128, 8, 128), np.float32)
    for ec in range(8):
        for gg in range(4):
            for k in range(8):
                fc = gg * 8 + k
                w2p[:, ec, gg, :, k, :] = w2T[
                    :, fc * 128 : (fc + 1) * 128, ec * 128 : (ec + 1) * 128
                ]
    g["w2_pan"] = _to_bf16(np.ascontiguousarray(w2p))
    g["wv_pan"] = _to_fp8(np.ascontiguousarray(wvT.reshape(L, 8, 128, V_E)), EWV)
    # vbias enters the V PSUM group: layer 0 scale 2^(EWV+EX0), fused 2^(EWV+EX)
    vb_scaled = vbias.copy()
    vb_scaled[0] *= 2.0 ** (EWV + EX0)
    vb_scaled[1:] *= 2.0 ** (EWV + EX)
    g["vbias"] = _to_bf16(vb_scaled)
    g["borow"] = _to_bf16(
        np.asarray(inputs["bo"], np.float32).reshape(L, 1, D) * (2.0 ** (EO + EWO))
    )

    par = np.zeros((L, 128, 96), np.float32)
    par[:, :, PC_BQ : PC_BQ + 8] = (bqkv[:, 0:D] * sc).reshape(L, 8, 128).transpose(0, 2, 1)
    par[:, :, PC_BK : PC_BK + 8] = bqkv[:, D : 2 * D].reshape(L, 8, 128).transpose(0, 2, 1)
    for l, v in par_bq_override:
        par[l, :, PC_BQ : PC_BQ + 8] = v.reshape(8, 128).T
    for l, v in par_bk_override:
        par[l, :, PC_BK : PC_BK + 8] = v.reshape(8, 128).T
    par[:, :, PC_BQ : PC_BQ + 8] *= 2.0 ** EQ
    par[:, :, PC_BK : PC_BK + 8] *= 2.0 ** EK
    par[:, :, PC_BO : PC_BO + 8] = np.asarray(inputs["bo"], np.float32).reshape(L, 8, 128).transpose(0, 2, 1)
    par[:, :, PC_B2 : PC_B2 + 8] = np.asarray(inputs["b2"], np.float32).reshape(L, 8, 128).transpose(0, 2, 1)
    par[:, :, PC_L1G : PC_L1G + 8] = np.asarray(inputs["g1"], np.float32).reshape(L, 8, 128).transpose(0, 2, 1)
    par[:, :, PC_L1B : PC_L1B + 8] = np.asarray(inputs["be1"], np.float32).reshape(L, 8, 128).transpose(0, 2, 1)
    par[:, :, PC_L2G : PC_L2G + 8] = np.asarray(inputs["g2"], np.float32).reshape(L, 8, 128).transpose(0, 2, 1)
    par[:, :, PC_L2B : PC_L2B + 8] = np.asarray(inputs["be2"], np.float32).reshape(L, 8, 128).transpose(0, 2, 1)
    par[:, :, PC_B1 : PC_B1 + 32] = np.asarray(inputs["b1"], np.float32).reshape(L, 32, 128).transpose(0, 2, 1)
    g["par"] = par

    g["ones1"] = np.ones((1, 128), np.float32)
    g["oneso"] = np.full((1, 128), 2.0 ** (EO - EK), np.float32)
    g["onesd"] = np.full((128, 1), 1.0 / D, np.float32)
    g["onesrb"] = _to_bf16(np.ones((1, 512), np.float32))
    # mask matmul runs as fp8-DR: diag(240) identity pair (slot1 zero) times
    # a -96-valued kill mask gives 224*-96 = -21504 in score PSUM; with the
    # exp scale 2^-(EQ+EK) that is -21, enough to zero any masked prob.
    idz = np.zeros((128, 2, 128), np.float32)
    idz[:, 0, :] = np.eye(128, dtype=np.float32) * 224.0
    g["id128"] = _to_fp8(idz, 0)
    maskt = np.zeros((128, MASKW), np.float32)
    p_ = np.arange(128)[:, None]
    jj = np.arange(512)[None, :]
    for c, slot in MASKS.items():
        kill = jj < (p_ + c * 128 - 576)
        maskt[:, slot * 512 : (slot + 1) * 512] = np.where(kill, -96.0, 0.0)
    g["maskt"] = _to_fp8(maskt, 0)

    xb = np.asarray(inputs["x"], np.float32).transpose(1, 0, 2)
    in_maps = []
    for c in range(8):
        b, hh = c // 2, c % 2
        pb = np.full((NC, 128, 1), EP * LN2_, np.float32)
        if hh == 0:
            pb[0:4] = NEG
            pb[8, 64:128] = NEG
        else:
            pb[8] = NEG
        m = dict(g)
        m["x0"] = np.ascontiguousarray(xb[b, hh * T : (hh + 1) * T, :].T)
        m["pb"] = pb
        in_maps.append(m)
    return in_maps


def kernel(**inputs):
    from concourse.bass_utils import run_bass_kernel_spmd

    if "nc" not in _CACHE:
        _CACHE["nc"] = _build_program()
    nc = _CACHE["nc"]
    in_maps = _host_prep(inputs)
    res = run_bass_kernel_spmd(nc, in_maps, core_ids=list(range(8)))
    out = np.zeros((S, B, D), np.float32)
    for c in range(8):
        b, hh = c // 2, c % 2
        out[hh * T : (hh + 1) * T, b, :] = res.results[c]["y"].T
    return out
